# revision 4
# baseline (speedup 1.0000x reference)
"""Causal self-attention (B=2, T=2048, C=1024, H=16) on 8 NeuronCores.

Sharding: 2 heads per core (tensor parallel over heads), both batch elements
on every core. Host-side: builds x^T and per-core weight slices, casts to
bf16; device computes its heads' attention and a partial output projection;
host sums the 8 partials and adds the bias.

Device-side layout is fully transposed so no on-chip transposes are needed:
  Q^T,K^T [d, t]  <- Wqkv-slice stationary  @ x^T moving
  V [t, d]        <- x^T-tile stationary    @ Wqkv-v moving   (natural layout)
  S^T [j, i]      <- K^T stationary (2 heads row-packed, D=64 contraction)
  E = exp(S/8)    <- ScalarE, PSUM -> bf16 SBUF, diagonal tiles masked on DVE
  O^T_ext [65, i] <- [V | ones] stationary @ E^T moving (row 64 = softmax denom)
  y_partial       <- O^T stationary @ Wo-slice moving, / denom via recip+bcast
"""

import os
import sys

for _p in ("/root/.axon_site/_ro/trn_rl_repo", "/opt/trn_rl_repo"):
    if os.path.isdir(_p) and _p not in sys.path:
        sys.path.append(_p)

import numpy as np
import ml_dtypes

import concourse.bass as bass
import concourse.tile as tile
from concourse import bacc, mybir
from concourse.bass_utils import run_bass_kernel_spmd

B, T, C, H, D = 2, 2048, 1024, 16, 64
NCORES = 8
HPC = H // NCORES          # heads per core = 2
BT = B * T                 # 4096 tokens
CI = C // 128              # 8 contraction sub-tiles
IT = T // 512              # 4 query tiles per batch
JT = T // 128              # 16 key tiles per batch

BF16 = mybir.dt.bfloat16
F32 = mybir.dt.float32
OUT_DT = np.float32        # partial-sum dtype sent back to host

_CACHE = {}


def _build():
    nc = bacc.Bacc("TRN2", target_bir_lowering=False, debug=False,
                   enable_asserts=True, num_devices=NCORES)

    xt_d = nc.dram_tensor("xt", [C, BT], BF16, kind="ExternalInput")
    wqkv_d = nc.dram_tensor("wqkv", [C, 3 * 128], BF16, kind="ExternalInput")
    wo_d = nc.dram_tensor("wo", [128, C], BF16, kind="ExternalInput")
    mask_d = nc.dram_tensor("mask", [128, 4 * 512], BF16, kind="ExternalInput")
    y_d = nc.dram_tensor("y", [BT, C], mybir.dt.from_np(OUT_DT), kind="ExternalOutput")

    with tile.TileContext(nc) as tc:
        with (
            tc.tile_pool(name="const", bufs=1) as const_pool,
            tc.tile_pool(name="qkt", bufs=1) as qkt_pool,
            tc.tile_pool(name="vsb", bufs=1) as v_pool,
            tc.tile_pool(name="ot", bufs=1) as ot_pool,
            tc.tile_pool(name="e", bufs=20) as e_pool,
            tc.tile_pool(name="small", bufs=4) as small_pool,
            tc.tile_pool(name="yout", bufs=4) as yout_pool,
            tc.tile_pool(name="ps_qkv", bufs=2, space="PSUM") as ps_qkv,
            tc.tile_pool(name="ps_s", bufs=2, space="PSUM") as ps_s,
            tc.tile_pool(name="ps_o", bufs=2, space="PSUM") as ps_o,
            tc.tile_pool(name="ps_y", bufs=2, space="PSUM") as ps_y,
        ):
            # ---- load constants -------------------------------------------
            xt_sb = const_pool.tile([128, CI, BT], BF16)
            xt_ap = xt_d.ap().rearrange("(o p) t -> p o t", p=128)
            for ci in range(CI):
                nc.sync.dma_start(xt_sb[:, ci:ci + 1, :], xt_ap[:, ci:ci + 1, :])
            wqkv_sb = const_pool.tile([128, CI, 3 * 128], BF16)
            nc.sync.dma_start(wqkv_sb[:], wqkv_d.ap().rearrange("(o p) n -> p o n", p=128))
            wo_sb = const_pool.tile([128, C], BF16)
            nc.sync.dma_start(wo_sb[:], wo_d.ap())
            mask_sb = const_pool.tile([128, 4, 512], BF16)
            nc.sync.dma_start(mask_sb[:], mask_d.ap().rearrange("p (m i) -> p m i", m=4))

            qt_sb = [qkt_pool.tile([128, T], BF16, name=f"qt{b}") for b in range(B)]
            kt_sb = [qkt_pool.tile([128, T], BF16, name=f"kt{b}") for b in range(B)]
            # V in natural layout with a ones column per head: [vh0 | 1 | vh1 | 1]
            v_sb = [v_pool.tile([128, JT, 130], BF16, name=f"v{b}") for b in range(B)]
            ot_sb = [ot_pool.tile([128, T], BF16, name=f"ot{b}") for b in range(B)]

            for b in range(B):
                nc.vector.memset(v_sb[b][:, :, 64:65], 1.0)
                nc.vector.memset(v_sb[b][:, :, 129:130], 1.0)

            for b in range(B):
                t0 = b * T
                # ---- Q^T / K^T projection ([128 cols, 512] psum tiles) ----
                for g, dst in ((0, qt_sb[b]), (1, kt_sb[b])):
                    for t4 in range(IT):
                        ps = ps_qkv.tile([128, 512], F32, tag="qkv")
                        for ci in range(CI):
                            nc.tensor.matmul(
                                ps[:],
                                wqkv_sb[:, ci, g * 128:(g + 1) * 128],
                                xt_sb[:, ci, t0 + t4 * 512: t0 + (t4 + 1) * 512],
                                start=(ci == 0), stop=(ci == CI - 1),
                            )
                        nc.vector.tensor_copy(dst[:, t4 * 512:(t4 + 1) * 512], ps[:])
                # ---- V projection (natural layout) ------------------------
                for jt in range(JT):
                    psv = ps_qkv.tile([128, 512], F32, tag="qkv", name="psv")
                    for ci in range(CI):
                        nc.tensor.matmul(
                            psv[:, 0:128],
                            xt_sb[:, ci, t0 + jt * 128: t0 + (jt + 1) * 128],
                            wqkv_sb[:, ci, 256:384],
                            start=(ci == 0), stop=(ci == CI - 1),
                        )
                    dst = v_sb[b][:, jt, :].rearrange("p (h x) -> p h x", h=2)[:, :, 0:64]
                    nc.vector.tensor_copy(dst, psv[:, 0:128].rearrange("p (h x) -> p h x", h=2))

                # ---- attention per 512-query tile -------------------------
                for it in range(IT):
                    jn = 4 * (it + 1)
                    i_sl = slice(it * 512, (it + 1) * 512)
                    es = []
                    for jt in range(jn):
                        for h in range(2):
                            hs = slice(h * 64, (h + 1) * 64)
                            ps = ps_s.tile([128, 512], F32, tag="s")
                            nc.tensor.matmul(
                                ps[:],
                                kt_sb[b][hs, jt * 128:(jt + 1) * 128],
                                qt_sb[b][hs, i_sl],
                                start=True, stop=True,
                                tile_position=(h * 64, 0),
                            )
                            e = e_pool.tile([128, 512], BF16, tag="e")
                            nc.scalar.activation(
                                e[:], ps[:], mybir.ActivationFunctionType.Exp,
                                scale=0.125,
                            )
                            if jt >= 4 * it:
                                nc.vector.tensor_mul(e[:], e[:], mask_sb[:, jt - 4 * it, :])
                            es.append(e)
                    op = [ps_o.tile([65, 512], F32, tag="o", name=f"o{b}_{it}_{h}")
                          for h in range(2)]
                    for jt in range(jn):
                        for h in range(2):
                            nc.tensor.matmul(
                                op[h][:],
                                v_sb[b][:, jt, h * 65:(h + 1) * 65],
                                es[2 * jt + h][:],
                                start=(jt == 0), stop=(jt == jn - 1),
                            )
                    for h in range(2):
                        rc = small_pool.tile([1, 512], F32, tag="rc")
                        nc.vector.reciprocal(rc[:], op[h][64:65, :])
                        bc = small_pool.tile([64, 512], F32, tag="bc")
                        nc.gpsimd.partition_broadcast(bc[:], rc[:])
                        nc.vector.tensor_mul(
                            ot_sb[b][h * 64:(h + 1) * 64, i_sl], op[h][0:64, :], bc[:]
                        )

                    # ---- output projection for this query tile ------------
                    for tt in range(4 * it, 4 * (it + 1)):
                        for n2 in range(2):
                            psy = ps_y.tile([128, 512], F32, tag="y")
                            nc.tensor.matmul(
                                psy[:],
                                ot_sb[b][:, tt * 128:(tt + 1) * 128],
                                wo_sb[:, n2 * 512:(n2 + 1) * 512],
                                start=True, stop=True,
                            )
                            yo = yout_pool.tile([128, 512], mybir.dt.from_np(OUT_DT), tag="yo")
                            nc.vector.tensor_copy(yo[:], psy[:])
                            nc.sync.dma_start(
                                y_d.ap()[t0 + tt * 128: t0 + (tt + 1) * 128,
                                         n2 * 512:(n2 + 1) * 512],
                                yo[:],
                            )

    nc.compile()
    return nc


def _prep_core_inputs(x, Wqkv, Wo):
    """Host-side slicing/packing. Returns list of per-core input dicts."""
    xt = np.ascontiguousarray(x.reshape(BT, C).T).astype(ml_dtypes.bfloat16)

    # causal masks for the 4 diagonal 128x512 sub-blocks
    j = np.arange(128)[:, None]
    i = np.arange(512)[None, :]
    masks = np.concatenate(
        [(j <= i - 128 * m) for m in range(4)], axis=1
    ).astype(ml_dtypes.bfloat16)

    in_maps = []
    for c in range(NCORES):
        h0, h1 = HPC * c, HPC * c + 1
        cols = []
        for g in range(3):  # q, k, v blocks of Wqkv
            for h in (h0, h1):
                cols.append(Wqkv[:, g * C + h * D:(g * C + (h + 1) * D)])
        wqkv_l = np.ascontiguousarray(np.concatenate(cols, axis=1)).astype(ml_dtypes.bfloat16)
        wo_l = np.ascontiguousarray(
            Wo[h0 * D:(h1 + 1) * D, :]
        ).astype(ml_dtypes.bfloat16)
        in_maps.append({"xt": xt, "wqkv": wqkv_l, "wo": wo_l, "mask": masks})
    return in_maps


def kernel(x, Wqkv, Wo, bo):
    x = np.asarray(x, dtype=np.float32)
    Wqkv = np.asarray(Wqkv, dtype=np.float32)
    Wo = np.asarray(Wo, dtype=np.float32)
    bo = np.asarray(bo, dtype=np.float32)

    nc = _CACHE.get("nc")
    if nc is None:
        nc = _CACHE["nc"] = _build()

    in_maps = _prep_core_inputs(x, Wqkv, Wo)
    res = run_bass_kernel_spmd(nc, in_maps, core_ids=list(range(NCORES)),
                               **_CACHE.get("run_kwargs", {}))
    _CACHE["last_result"] = res

    y = np.zeros((BT, C), dtype=np.float64)
    for c in range(NCORES):
        y += res.results[c]["y"].astype(np.float64)
    y = (y + bo.astype(np.float64)).astype(np.float32)
    return y.reshape(B, T, C)
